# revision 23
# baseline (speedup 1.0000x reference)
"""GAT layer (4 heads, mean-combined) on 8 Trainium2 NeuronCores.

v7 strategy (single SPMD program; per-core variation lives in input data):
  - Edges sharded by dst range: core m owns dst in [12500*m, 12500*(m+1)).
  - out[v] = mean_k (sum_e e_k*hw_k[src_e]) / (sum_e e_k) over edges with
    dst==v, e_k = clip(exp(lrelu(s_src+s_dst)), lo, hi). No softmax gather.
  - Phase 1 (bf16): haug = h @ [Wsi | W_k a1_k | W_k a2_k] -> DRAM table
    tabA[node] = [ones(4) | hw interleaved (256: col 4+4j+k = W_k col j)
    | s_src(4) | pad] (768B rows). s_dst for the core's own dst range is
    built per window (hTo chunk loads) and stays resident in SBUF (s2st).
    p1a chunks interleave 1-per-2 phase-2 chunks; casts alternate V/S.
  - Phase 2 runs as 4 block-major streams (block = 32768 src rows, the
    int16 gather index range). Each chunk (<=3 windows) gathers all its
    edge rows with ONE dma_gather (768B rows, idx sorted by src),
    round-robined over the 4 SWDGE queues (queue q runs on Q7 core pair
    (2q,2q+1) -> concurrent descriptor generation). Indicator matrices
    Ind[slot,rank] and IndT[rank,slot] are precomputed on the host and
    streamed from DRAM as one interleaved stream (one DMA per chunk).
    psE = IndT_j.T @ swin + Ident.T @ s_src_j accumulates the full logit
    L per slot in PSUM on the PE; ScalarE computes exp(L) and exp(0.2L);
    e = min(max of the two, hi) (low clip provably never binds).
    One fused DVE multiply Gw[s,0:260] = G[s,0:260] * rep65(e[s,0:4])
    weights all 4 heads AND materializes e into cols 0:4 (ones*e) for
    the denominators; segment-sums via PSUM[rank,260] += Ind_j.T @ Gw_j
    accumulate into an SBUF-resident acc[128, 98*260] (bf16).
  - Final: batched normalization in 3 window groups (big strided DVE
    ops over acc), bf16 output rows, one DMA per group; host casts f32.
  - acc layout per window: [0:4 denom | 4+4j+k msg(j,k)].
No scatter primitives and no collectives are needed.
"""
import os

import numpy as np
import ml_dtypes

import concourse.bass as bass
import concourse.bacc as bacc
import concourse.mybir as mybir
from concourse import tile
from concourse.bass import broadcast_tensor_aps
from concourse.bass_utils import run_bass_kernel_spmd

N = 100000
E = 1600000
IN_DIM = 128
OUT_DIM = 64
HEADS = 4
SLOPE = 0.2
CLIP_LO, CLIP_HI = 0.005, 10.0

NC = 8
NPC = N // NC            # 12500 dst nodes per core
WINR = 128               # ranks (local dst nodes) per window
NWIN = -(-NPC // WINR)   # 98 windows per core
NLOC = NWIN * WINR       # 12544 padded local nodes
NPAD = 49 * 2048         # 100352: padded node count for phase 1 tiling
SBLK = 32768             # src block size (int16 gather index limit)
NBLK = -(-NPAD // SBLK)  # 4
CPB = SBLK // 2048       # phase-1 chunks per block (16)
BROW = 384               # tabA row in bf16 elems: 4 ones | 256 hw | 4 s1 | pad
USED = 260               # weighted row elems: 4 denom | 256 msg
BF = ml_dtypes.bfloat16
F32 = mybir.dt.float32
BF16 = mybir.dt.bfloat16
I16 = mybir.dt.int16
ADD, MUL, MIN, MAX = (
    mybir.AluOpType.add,
    mybir.AluOpType.mult,
    mybir.AluOpType.min,
    mybir.AluOpType.max,
)


def _split_waits(nc):
    """This walrus build rejects >1 sync-wait per instruction. Hoist extras
    onto same-engine nops inserted immediately before the owner."""
    n = 0
    for f in nc.m.functions:
        for bb in f.blocks:
            new_list = []
            for ins in bb.instructions:
                si = ins.sync_info
                if si is not None and si.on_wait and len(si.on_wait) > 1:
                    waits = list(si.on_wait)
                    si.on_wait = waits[-1:]
                    for w in waits[:-1]:
                        nop = mybir.InstNoOp(
                            name=nc.get_next_instruction_name(),
                            engine=ins.engine,
                            sync_info=mybir.SyncInfo(on_wait=[w], on_update=[]),
                            bass_nofuse=True,
                        )
                        nc.register_instruction(nop)
                        new_list.append(nop)
                        n += 1
                new_list.append(ins)
            bb.instructions[:] = new_list
    return n


def _offsets(WS):
    """Block-major subtile offsets. WS is [NWIN][NBLK]; subtiles are laid
    out in (b, w) order. Returns suboff[b][w] = global subtile index of
    region (b, w) and TOTSUB."""
    suboff = [[0] * NWIN for _ in range(NBLK)]
    acc = 0
    for b in range(NBLK):
        for w in range(NWIN):
            suboff[b][w] = acc
            acc += WS[w][b]
    return suboff, acc


def _preprocess(h, edges, W, a):
    h = np.asarray(h, np.float32)
    W = np.asarray(W, np.float32)
    a = np.asarray(a, np.float32)
    src = np.asarray(edges[0], np.int64)
    dst = np.asarray(edges[1], np.int64)

    # Ws cols: 0:256 interleaved hw (col 4j+k = W_k col j), 256:260 s1,
    # 260:264 s2
    Ws = np.zeros((IN_DIM, 264), np.float32)
    for k in range(HEADS):
        Ws[:, k:256:4] = W[k]
        Ws[:, 256 + k] = W[k] @ a[k, :OUT_DIM]
        Ws[:, 260 + k] = W[k] @ a[k, OUT_DIM:]
    ident = np.eye(128, dtype=BF)

    hpad = np.zeros((NPAD, IN_DIM), np.float32)
    hpad[:N] = h
    hT = np.ascontiguousarray(hpad.T.astype(BF))  # [128, NPAD] bf16

    # per-core own-range h, transposed, padded to NLOC
    hTo = np.zeros((NC, IN_DIM, NLOC), BF)
    for m in range(NC):
        lo = m * NPC
        hi = min(lo + NLOC, N)
        hTo[m, :, : hi - lo] = h[lo:hi].T.astype(BF)

    m = dst // NPC
    dl = dst - m * NPC
    w = dl // WINR
    rr = dl - w * WINR
    blk = src // SBLK
    lsrc = src - blk * SBLK

    # static subtile capacity per (window, block): max over cores
    key = (m * NWIN + w) * NBLK + blk
    cnt = np.bincount(key, minlength=NC * NWIN * NBLK).reshape(NC, NWIN, NBLK)
    MC = cnt.max(axis=0).astype(np.int64)                    # [NWIN, NBLK]
    WS = (-(-MC // 128)).astype(np.int64)                    # [NWIN, NBLK]
    suboff, TOTSUB = _offsets(WS.tolist())
    suboff_a = np.asarray(suboff)                            # [NBLK, NWIN]

    # slot id for each edge: block-major (b, w) regions, sorted by src id
    order = np.lexsort((lsrc, w, blk, m))
    wo, mo = w[order], m[order]
    rro, blko, lsrco = rr[order], blk[order], lsrc[order]
    keyo = (mo * NBLK + blko) * NWIN + wo
    cnt_bm = np.bincount(
        (m * NBLK + blk) * NWIN + w, minlength=NC * NBLK * NWIN
    )
    starts = np.zeros(NC * NBLK * NWIN, np.int64)
    starts[1:] = np.cumsum(cnt_bm)[:-1]
    within = np.arange(E) - starts[keyo]
    regbase = suboff_a[blko, wo] * 128
    slot = regbase + within                                  # within-core slot

    gidx = np.zeros((NC, TOTSUB * 128), np.int16)
    rri = np.full((NC, TOTSUB * 128), 128, np.int64)         # 128 = invalid
    gidx[mo, slot] = lsrco.astype(np.int16)
    rri[mo, slot] = rro

    # wrap16 packing for int16 idx streams ([16, n/16] tiled to 128 parts)
    t = gidx.reshape(NC, -1, 16).transpose(0, 2, 1)
    gidx_p = np.ascontiguousarray(np.tile(t, (1, 8, 1)))     # [NC,128,TOTSUB*8]

    # host-built indicator streams (bf16 one-hot; rank 128 -> zero row),
    # Ind and IndT interleaved per subtile: cols [s*256:s*256+128] = Ind_s
    # ([slot_p, rank_c]), [s*256+128:(s+1)*256] = IndT_s ([rank_p, slot_c])
    eye129 = np.zeros((129, 128), BF)
    eye129[:128] = np.eye(128, dtype=BF)
    # rr_sl[m, p, s] = rank of slot s*128+p
    rr_sl = rri.reshape(NC, TOTSUB, 128).transpose(0, 2, 1)

    shared = {
        "hT": hT,
        "Ws": np.ascontiguousarray(Ws.astype(BF)),
        "ident": ident,
    }
    per_core = []
    for m_ in range(NC):
        A = eye129[rr_sl[m_]]                  # [128slot, TOTSUB, 128rank]
        C = np.empty((128, TOTSUB, 256), BF)
        C[:, :, 0:128] = A
        C[:, :, 128:256] = A.transpose(2, 1, 0)
        per_core.append(
            {
                "hTo": np.ascontiguousarray(hTo[m_]),
                "gidx": gidx_p[m_],
                "indc": C.reshape(128, TOTSUB * 256),
            }
        )
    return shared, per_core, WS.tolist(), MC.tolist()


CG = 3                   # windows per chunk (one gather / DVE op group)
NGRP = 3                 # window groups for the batched final normalize


def _build(WS, MC):
    suboff, TOTSUB = _offsets(WS)
    # chunks: (b, w0, w1) with w1-w0 <= CG
    chunks = []
    for b in range(NBLK):
        w0 = 0
        while w0 < NWIN:
            w1 = min(w0 + CG, NWIN)
            chunks.append((b, w0, w1))
            w0 = w1
    CWSMAX = max(
        sum(WS[w][b] for w in range(w0, w1)) for b, w0, w1 in chunks
    )
    boffg = [suboff[b][0] for b in range(NBLK)] + [TOTSUB]
    BSUBMAX = max(boffg[b + 1] - boffg[b] for b in range(NBLK))
    # first non-empty block per window
    first_b = [
        min(b for b in range(NBLK) if WS[w][b] > 0) for w in range(NWIN)
    ]
    GRPB = [round(g * NWIN / NGRP) for g in range(NGRP + 1)]
    GMAX = max(GRPB[g + 1] - GRPB[g] for g in range(NGRP))

    nc = bacc.Bacc(None, num_swdge_queues=4)
    hT_d = nc.declare_dram_parameter("hT", [IN_DIM, NPAD], BF16, isOutput=False)
    hTo_d = nc.declare_dram_parameter("hTo", [IN_DIM, NLOC], BF16, isOutput=False)
    Ws_d = nc.declare_dram_parameter("Ws", [IN_DIM, 264], BF16, isOutput=False)
    ident_d = nc.declare_dram_parameter("ident", [128, 128], BF16, isOutput=False)
    gidx_d = nc.declare_dram_parameter(
        "gidx", [128, TOTSUB * 8], I16, isOutput=False
    )
    indc_d = nc.declare_dram_parameter(
        "indc", [128, TOTSUB * 256], BF16, isOutput=False
    )
    out_d = nc.declare_dram_parameter("out", [NLOC, OUT_DIM], BF16, isOutput=True)

    # one table per gather block so block-b gathers only wait on their
    # own quarter of phase 1 (Tile tracks DRAM deps per tensor)
    tabs = [
        nc.dram_tensor(f"tabA{b}", [min(SBLK, NPAD - SBLK * b), BROW], BF16)
        for b in range(NBLK)
    ]

    with tile.TileContext(nc) as tc:
        with (
            tc.tile_pool(name="const", bufs=1) as cpool,
            tc.tile_pool(name="pgi", bufs=2) as pgi,
            tc.tile_pool(name="pind", bufs=2) as pind,
            tc.tile_pool(name="pG", bufs=3) as pG,
            tc.tile_pool(name="pGw", bufs=2) as pGw,
            tc.tile_pool(name="p1h", bufs=2) as p1h,
            tc.tile_pool(name="p1b", bufs=2) as p1b,
            tc.tile_pool(name="p1ps", bufs=2, space="PSUM") as p1ps,
            tc.tile_pool(name="p1st", bufs=2) as p1st,
            tc.tile_pool(name="pe", bufs=3) as pe,
            tc.tile_pool(name="pps", bufs=3, space="PSUM") as pps,
            tc.tile_pool(name="ppsE", bufs=1, space="PSUM") as ppsE,
            tc.tile_pool(name="pno", bufs=1) as pno,
        ):
            Ws_t = cpool.tile_from(Ws_d[:])
            ident_t = cpool.tile_from(ident_d[:])
            s2st = cpool.tile([128, NWIN * 4], BF16)
            acc = cpool.tile([128, NWIN * USED], BF16)

            def load_block_idx(b):
                g0, g1 = boffg[b], boffg[b + 1]
                gi = pgi.tile([128, BSUBMAX * 8], I16, tag="gi")
                nc.sync.dma_start(
                    gi[:, 0 : (g1 - g0) * 8], gidx_d[:, g0 * 8 : g1 * 8]
                )
                return gi

            gi_b = load_block_idx(0)
            for _ in range(3):
                g0t = pG.tile([128, CWSMAX * BROW], BF16, tag="G")
                nc.gpsimd.memset(g0t[:], 0.0)
            # one-time: ones into cols 0:4 of each 16-row stb buffer
            for _ in range(2):
                stb0 = p1st.tile([128, 16 * BROW], BF16, tag="stb")
                nc.gpsimd.memset(
                    stb0[:].rearrange("p (i e) -> p i e", e=BROW)[:, :, 0:4],
                    1.0,
                )

            # ---- phase 1b: s_dst for own dst range -> SBUF (s2st) ----
            for t2 in range(NWIN):
                htw = p1b.tile([128, 128], BF16, tag="htw")
                nc.sync.dma_start(htw[:], hTo_d[:, t2 * 128 : (t2 + 1) * 128])
                ps2t = ppsE.tile([128, CWSMAX * 12], F32, tag="psE", name="psE")
                ps2 = ps2t[:, 0:4]
                nc.tensor.matmul(
                    ps2,
                    htw[:],
                    Ws_t[:, 260:264],
                    start=True,
                    stop=True,
                )
                nc.scalar.activation(
                    s2st[:, t2 * 4 : (t2 + 1) * 4],
                    ps2,
                    mybir.ActivationFunctionType.Copy,
                )

            # ---- phase 1a chunk: haug = h @ Ws -> tabA (bf16 rows) ----
            def p1a_chunk(c):
                hc = p1h.tile([128, 2048], BF16, tag="hc")
                nc.sync.dma_start(hc[:], hT_d[:, c * 2048 : (c + 1) * 2048])
                stb = p1st.tile([128, 16 * BROW], BF16, tag="stb")
                for i in range(16):
                    ps = p1ps.tile([128, USED], F32, tag="ps")
                    nc.tensor.matmul(
                        ps[:],
                        hc[:, i * 128 : (i + 1) * 128],
                        Ws_t[:, 0:USED],
                        start=True,
                        stop=True,
                    )
                    # row: [0:4 ones (persist) | 4:260 hw | 260:264 s1]
                    dst_sl = stb[:, i * BROW + 4 : i * BROW + 264]
                    if i % 2 == 0:
                        nc.vector.tensor_copy(dst_sl, ps[:])
                    else:
                        nc.scalar.activation(
                            dst_sl, ps[:], mybir.ActivationFunctionType.Copy
                        )
                cb, co = c // CPB, (c % CPB) * 2048
                nc.sync.dma_start(
                    tabs[cb][co : co + 2048, :].rearrange(
                        "(i p) e -> p i e", p=128
                    ),
                    stb[:].rearrange("p (i e) -> p i e", e=BROW),
                )

            for c in range(CPB):
                p1a_chunk(c)
            pc = CPB

            # ---- phase 2: block-major chunked streams, interleaved with
            # the remaining phase-1a chunks (1 per 2 phase-2 chunks) ----
            cur_b = 0
            ci = 0
            qc = [0]
            for b, w0, w1 in chunks:
                if b != cur_b:
                    gi_b = load_block_idx(b)
                    cur_b = b
                wss = [WS[w][b] for w in range(w0, w1)]
                cws = sum(wss)
                if cws == 0:
                    continue
                j0 = suboff[b][w0]
                jl = j0 - boffg[b]          # local subtile offset in block

                # indicator streams for this chunk from DRAM (interleaved)
                Ic = pind.tile([128, CWSMAX * 256], BF16, tag="Ic")
                nc.sync.dma_start(
                    Ic[:, 0 : cws * 256],
                    indc_d[:, j0 * 256 : (j0 + cws) * 256],
                )
                Ic4 = Ic[:, 0 : cws * 256].rearrange(
                    "p (s t c) -> p s t c", t=2, c=128
                )
                Ind3 = Ic4[:, :, 0, :]
                IndT3 = Ic4[:, :, 1, :]

                # gather per-edge rows (one call per region), round-robin
                # across the 4 SWDGE queues: queue q runs on Q7 core pair
                # (2q, 2q+1), so gathers on different queues generate
                # descriptors concurrently. Calls stay per-window to keep
                # each descriptor batch within the SWDGE ring capacity.
                G = pG.tile([128, CWSMAX * BROW], BF16, tag="G")
                G3 = G[:, 0 : cws * BROW].rearrange("p (s e) -> p s e", e=BROW)
                jg = 0
                for wi in range(len(wss)):
                    ws = wss[wi]
                    if ws == 0:
                        continue
                    mc = MC[w0 + wi][b]
                    nc.gpsimd.dma_gather(
                        G3[:, jg : jg + ws, :],
                        tabs[b][:, :],
                        gi_b[:, (jl + jg) * 8 : (jl + jg + ws) * 8],
                        mc,
                        mc,
                        BROW,
                        queue_num=qc[0] % 4,
                    )
                    qc[0] += 1
                    jg += ws

                # psE[slot, 4] = IndT_j.T @ swin(owner) + Ident.T @ s1_j
                # == full logit L per slot, accumulated on the PE
                psEt = ppsE.tile([128, CWSMAX * 12], F32, tag="psE", name="psE")
                pb0 = (ci % 3) * CWSMAX * 4
                ci += 1
                psE = psEt[:, pb0 : pb0 + CWSMAX * 4]
                j = 0
                for wi, w in enumerate(range(w0, w1)):
                    for _ in range(wss[wi]):
                        nc.tensor.matmul(
                            psE[:, 4 * j : 4 * j + 4],
                            IndT3[:, j, :],
                            s2st[:, 4 * w : 4 * w + 4],
                            start=True,
                            stop=False,
                        )
                        nc.tensor.matmul(
                            psE[:, 4 * j : 4 * j + 4],
                            ident_t[:],
                            G3[:, j, 260:264],
                            start=False,
                            stop=True,
                        )
                        j += 1

                # e = min(max(exp(L), exp(0.2 L)), HI); lo-clip never
                # binds (|logit| <= ~22 -> e >= 0.01)
                E1 = pe.tile([128, CWSMAX * 4], F32, tag="E1")
                nc.scalar.activation(
                    E1[:, 0 : cws * 4],
                    psE[:, 0 : cws * 4],
                    mybir.ActivationFunctionType.Exp,
                )
                E2 = pe.tile([128, CWSMAX * 4], F32, tag="E2")
                nc.scalar.activation(
                    E2[:, 0 : cws * 4],
                    psE[:, 0 : cws * 4],
                    mybir.ActivationFunctionType.Exp,
                    scale=SLOPE,
                )
                nc.vector.tensor_tensor(
                    E1[:, 0 : cws * 4],
                    E1[:, 0 : cws * 4],
                    E2[:, 0 : cws * 4],
                    MAX,
                )
                Eb = pe.tile([128, CWSMAX * 4], BF16, tag="Eb")
                nc.vector.tensor_scalar(
                    Eb[:, 0 : cws * 4], E1[:, 0 : cws * 4], CLIP_HI, None, MIN
                )

                # one fused weighting multiply:
                # Gw[s, 0:260] = G[s, 0:260] * rep65(e[s, 0:4])
                Gw = pGw.tile([128, CWSMAX * USED], BF16, tag="Gw")
                Gw4 = Gw[:, 0 : cws * USED].rearrange(
                    "p (s g f) -> p s g f", g=65, f=4
                )
                G4 = G[:, 0 : cws * BROW].rearrange(
                    "p (s e) -> p s e", e=BROW
                )[:, :, 0:USED].rearrange("p s (g f) -> p s g f", f=4)
                Eb4 = Eb[:, 0 : cws * 4].rearrange(
                    "p (s o f) -> p s o f", o=1, f=4
                )
                eb_b, g4_b = broadcast_tensor_aps(Eb4, G4)
                nc.vector.tensor_tensor(Gw4, g4_b, eb_b, MUL)
                Gw3 = Gw[:, 0 : cws * USED].rearrange(
                    "p (s e) -> p s e", e=USED
                )

                # per-window segment matmuls -> PSUM -> acc[w]
                j = 0
                for wi, w in enumerate(range(w0, w1)):
                    ws = wss[wi]
                    if ws == 0:
                        continue
                    aw = acc[:, w * USED : (w + 1) * USED]
                    ps = pps.tile([128, USED], F32, tag="mps")
                    for jj in range(j, j + ws):
                        nc.tensor.matmul(
                            ps[:],
                            Ind3[:, jj, :],
                            Gw3[:, jj, :],
                            start=(jj == j),
                            stop=(jj == j + ws - 1),
                        )
                    j += ws
                    if b == first_b[w]:
                        nc.vector.tensor_copy(aw, ps[:])
                    else:
                        nc.vector.tensor_tensor(aw, ps[:], aw, ADD)

                if pc < NPAD // 2048 and ci % 2 == 0:
                    p1a_chunk(pc)
                    pc += 1

            while pc < NPAD // 2048:
                p1a_chunk(pc)
                pc += 1

            # ---- batched final normalization over window groups ----
            acc3 = acc[:].rearrange("p (w e) -> p w e", e=USED)
            R = pno.tile([128, NWIN * 4], F32, tag="R")
            nc.vector.tensor_scalar(
                R[:].rearrange("p (w f) -> p w f", f=4),
                acc3[:, :, 0:4],
                1e-30,
                None,
                MAX,
            )
            R2 = pno.tile([128, NWIN * 4], F32, tag="R2")
            nc.vector.reciprocal(R2[:], R[:])
            nc.vector.tensor_scalar(R2[:], R2[:], 1.0 / HEADS, None, MUL)
            for g in range(NGRP):
                g0, g1 = GRPB[g], GRPB[g + 1]
                gw = g1 - g0
                accg = acc[:, g0 * USED : g1 * USED].rearrange(
                    "p (w e) -> p w e", e=USED
                )[:, :, 4:USED].rearrange("p w (j f) -> p w j f", f=4)
                r2g = R2[:, g0 * 4 : g1 * 4].rearrange(
                    "p (w o f) -> p w o f", o=1, f=4
                )
                Og = pno.tile([128, GMAX * OUT_DIM], BF16, tag="Og")
                Og3 = Og[:, 0 : gw * OUT_DIM].rearrange(
                    "p (w j) -> p w j", j=OUT_DIM
                )
                T = pno.tile([128, GMAX * OUT_DIM], BF16, tag="T")
                T3 = T[:, 0 : gw * OUT_DIM].rearrange(
                    "p (w j) -> p w j", j=OUT_DIM
                )
                for k in range(HEADS):
                    r2b, mb = broadcast_tensor_aps(
                        r2g[:, :, :, k], accg[:, :, :, k]
                    )
                    if k == 0:
                        nc.vector.tensor_tensor(Og3, mb, r2b, MUL)
                    else:
                        nc.vector.tensor_tensor(T3, mb, r2b, MUL)
                        nc.vector.tensor_tensor(Og3, Og3, T3, ADD)
                nc.sync.dma_start(
                    out_d[g0 * 128 : g1 * 128, :].rearrange(
                        "(w p) j -> p w j", p=128
                    ),
                    Og3,
                )

    nc.compile()
    _split_waits(nc)
    return nc


def kernel(h, edges, W, a):
    shared, per_core, WS, MC = _preprocess(h, edges, W, a)
    nc = _build(WS, MC)
    in_maps = [{**shared, **pc} for pc in per_core]
    r = run_bass_kernel_spmd(
        nc, in_maps, list(range(NC)), trace=bool(os.environ.get("GAT_TRACE"))
    )
    res = r.results
    global _last_results, _last_exec_ns, _last_bkr
    _last_results = res
    _last_exec_ns = r.exec_time_ns
    _last_bkr = r
    out = np.zeros((N, OUT_DIM), np.float32)
    for m in range(NC):
        out[m * NPC : (m + 1) * NPC] = res[m]["out"][:NPC].astype(np.float32)
    return out


# revision 27
# speedup vs baseline: 1.1924x; 1.1924x over previous
"""GAT layer (4 heads, mean-combined) on 8 Trainium2 NeuronCores.

v7 strategy (single SPMD program; per-core variation lives in input data):
  - Edges sharded by dst range: core m owns dst in [12500*m, 12500*(m+1)).
  - out[v] = mean_k (sum_e e_k*hw_k[src_e]) / (sum_e e_k) over edges with
    dst==v, e_k = clip(exp(lrelu(s_src+s_dst)), lo, hi). No softmax gather.
  - Phase 1 (bf16): haug = h @ [Wsi | W_k a1_k | W_k a2_k] -> DRAM table
    tabA[node] = [ones(4) | hw interleaved (256: col 4+4j+k = W_k col j)
    | s_src(4) | pad] (768B rows). s_dst for the core's own dst range is
    built per window (hTo chunk loads) and stays resident in SBUF (s2st).
    p1a chunks interleave 1-per-2 phase-2 chunks; casts alternate V/S.
  - Phase 2 runs as 4 block-major streams (block = 32768 src rows, the
    int16 gather index range). Each chunk (<=3 windows) gathers all its
    edge rows with ONE dma_gather (768B rows, idx sorted by src),
    round-robined over the 4 SWDGE queues (queue q runs on Q7 core pair
    (2q,2q+1) -> concurrent descriptor generation). Indicator matrices
    Ind[slot,rank] and IndT[rank,slot] are precomputed on the host and
    streamed from DRAM as one interleaved stream (one DMA per chunk).
    psE = IndT_j.T @ swin + Ident.T @ s_src_j accumulates the full logit
    L per slot in PSUM on the PE; ScalarE computes exp(L) and exp(0.2L);
    e = min(max of the two, hi) (low clip provably never binds).
    One fused DVE multiply Gw[s,0:260] = G[s,0:260] * rep65(e[s,0:4])
    weights all 4 heads AND materializes e into cols 0:4 (ones*e) for
    the denominators; segment-sums via PSUM[rank,260] += Ind_j.T @ Gw_j
    accumulate into an SBUF-resident acc[128, 98*260] (bf16).
  - Final: batched normalization in 3 window groups (big strided DVE
    ops over acc), bf16 output rows, one DMA per group; host casts f32.
  - acc layout per window: [0:4 denom | 4+4j+k msg(j,k)].
No scatter primitives and no collectives are needed.
"""
import os

import numpy as np
import ml_dtypes

import concourse.bass as bass
import concourse.bacc as bacc
import concourse.mybir as mybir
from concourse import tile
from concourse.bass import broadcast_tensor_aps
from concourse.bass_utils import run_bass_kernel_spmd

N = 100000
E = 1600000
IN_DIM = 128
OUT_DIM = 64
HEADS = 4
SLOPE = 0.2
CLIP_LO, CLIP_HI = 0.005, 10.0

NC = 8
NPC = N // NC            # 12500 dst nodes per core
WINR = 128               # ranks (local dst nodes) per window
NWIN = -(-NPC // WINR)   # 98 windows per core
NLOC = NWIN * WINR       # 12544 padded local nodes
NPAD = 49 * 2048         # 100352: padded node count for phase 1 tiling
SBLK = 32768             # src block size (int16 gather index limit)
NBLK = -(-NPAD // SBLK)  # 4
CPB = SBLK // 2048       # phase-1 chunks per block (16)
BROW = 384               # tabA row in bf16 elems: 4 ones | 256 hw | 4 s1 | pad
USED = 260               # weighted row elems: 4 denom | 256 msg
BF = ml_dtypes.bfloat16
F32 = mybir.dt.float32
BF16 = mybir.dt.bfloat16
I16 = mybir.dt.int16
ADD, MUL, MIN, MAX = (
    mybir.AluOpType.add,
    mybir.AluOpType.mult,
    mybir.AluOpType.min,
    mybir.AluOpType.max,
)


def _split_waits(nc):
    """This walrus build rejects >1 sync-wait per instruction. Hoist extras
    onto same-engine nops inserted immediately before the owner."""
    n = 0
    for f in nc.m.functions:
        for bb in f.blocks:
            new_list = []
            for ins in bb.instructions:
                si = ins.sync_info
                if si is not None and si.on_wait and len(si.on_wait) > 1:
                    waits = list(si.on_wait)
                    si.on_wait = waits[-1:]
                    for w in waits[:-1]:
                        nop = mybir.InstNoOp(
                            name=nc.get_next_instruction_name(),
                            engine=ins.engine,
                            sync_info=mybir.SyncInfo(on_wait=[w], on_update=[]),
                            bass_nofuse=True,
                        )
                        nc.register_instruction(nop)
                        new_list.append(nop)
                        n += 1
                new_list.append(ins)
            bb.instructions[:] = new_list
    return n


def _offsets(WS):
    """Block-major subtile offsets. WS is [NWIN][NBLK]; subtiles are laid
    out in (b, w) order. Returns suboff[b][w] = global subtile index of
    region (b, w) and TOTSUB."""
    suboff = [[0] * NWIN for _ in range(NBLK)]
    acc = 0
    for b in range(NBLK):
        for w in range(NWIN):
            suboff[b][w] = acc
            acc += WS[w][b]
    return suboff, acc


def _preprocess(h, edges, W, a):
    h = np.asarray(h, np.float32)
    W = np.asarray(W, np.float32)
    a = np.asarray(a, np.float32)
    src = np.asarray(edges[0], np.int64)
    dst = np.asarray(edges[1], np.int64)

    # Ws cols: 0:256 interleaved hw (col 4j+k = W_k col j), 256:260 s1,
    # 260:264 s2
    Ws = np.zeros((IN_DIM, 264), np.float32)
    for k in range(HEADS):
        Ws[:, k:256:4] = W[k]
        Ws[:, 256 + k] = W[k] @ a[k, :OUT_DIM]
        Ws[:, 260 + k] = W[k] @ a[k, OUT_DIM:]
    ident = np.eye(128, dtype=BF)

    hpad = np.zeros((NPAD, IN_DIM), np.float32)
    hpad[:N] = h
    hT = np.ascontiguousarray(hpad.T.astype(BF))  # [128, NPAD] bf16

    # per-core own-range h, transposed, padded to NLOC
    hTo = np.zeros((NC, IN_DIM, NLOC), BF)
    for m in range(NC):
        lo = m * NPC
        hi = min(lo + NLOC, N)
        hTo[m, :, : hi - lo] = h[lo:hi].T.astype(BF)

    m = dst // NPC
    dl = dst - m * NPC
    w = dl // WINR
    rr = dl - w * WINR
    blk = src // SBLK
    lsrc = src - blk * SBLK

    # static subtile capacity per (window, block): max over cores
    key = (m * NWIN + w) * NBLK + blk
    cnt = np.bincount(key, minlength=NC * NWIN * NBLK).reshape(NC, NWIN, NBLK)
    MC = cnt.max(axis=0).astype(np.int64)                    # [NWIN, NBLK]
    WS = (-(-MC // 128)).astype(np.int64)                    # [NWIN, NBLK]
    suboff, TOTSUB = _offsets(WS.tolist())
    suboff_a = np.asarray(suboff)                            # [NBLK, NWIN]

    # slot id for each edge: block-major (b, w) regions, sorted by src id
    order = np.lexsort((lsrc, w, blk, m))
    wo, mo = w[order], m[order]
    rro, blko, lsrco = rr[order], blk[order], lsrc[order]
    keyo = (mo * NBLK + blko) * NWIN + wo
    cnt_bm = np.bincount(
        (m * NBLK + blk) * NWIN + w, minlength=NC * NBLK * NWIN
    )
    starts = np.zeros(NC * NBLK * NWIN, np.int64)
    starts[1:] = np.cumsum(cnt_bm)[:-1]
    within = np.arange(E) - starts[keyo]
    regbase = suboff_a[blko, wo] * 128
    slot = regbase + within                                  # within-core slot

    gidx = np.zeros((NC, TOTSUB * 128), np.int16)
    rri = np.full((NC, TOTSUB * 128), 128, np.int64)         # 128 = invalid
    gidx[mo, slot] = lsrco.astype(np.int16)
    rri[mo, slot] = rro

    # wrap16 packing for int16 idx streams ([16, n/16] tiled to 128 parts)
    t = gidx.reshape(NC, -1, 16).transpose(0, 2, 1)
    gidx_p = np.ascontiguousarray(np.tile(t, (1, 8, 1)))     # [NC,128,TOTSUB*8]

    # host-built indicator streams (bf16 one-hot; rank 128 -> zero row),
    # Ind and IndT interleaved per subtile: cols [s*256:s*256+128] = Ind_s
    # ([slot_p, rank_c]), [s*256+128:(s+1)*256] = IndT_s ([rank_p, slot_c])
    eye129 = np.zeros((129, 128), BF)
    eye129[:128] = np.eye(128, dtype=BF)
    # rr_sl[m, p, s] = rank of slot s*128+p
    rr_sl = rri.reshape(NC, TOTSUB, 128).transpose(0, 2, 1)

    shared = {
        "hT": hT,
        "Ws": np.ascontiguousarray(Ws.astype(BF)),
        "ident": ident,
    }
    per_core = []
    for m_ in range(NC):
        A = eye129[rr_sl[m_]]                  # [128slot, TOTSUB, 128rank]
        C = np.empty((128, TOTSUB, 256), BF)
        C[:, :, 0:128] = A
        C[:, :, 128:256] = A.transpose(2, 1, 0)
        per_core.append(
            {
                "hTo": np.ascontiguousarray(hTo[m_]),
                "gidx": gidx_p[m_],
                "indc": C.reshape(128, TOTSUB * 256),
            }
        )
    return shared, per_core, WS.tolist(), MC.tolist()


CG = 3                   # windows per chunk (one gather / DVE op group)
NGRP = 3                 # window groups for the batched final normalize


def _build(WS, MC):
    suboff, TOTSUB = _offsets(WS)
    # chunks: (b, w0, w1) with w1-w0 <= CG
    chunks = []
    for b in range(NBLK):
        w0 = 0
        while w0 < NWIN:
            w1 = min(w0 + CG, NWIN)
            chunks.append((b, w0, w1))
            w0 = w1
    CWSMAX = max(
        sum(WS[w][b] for w in range(w0, w1)) for b, w0, w1 in chunks
    )
    boffg = [suboff[b][0] for b in range(NBLK)] + [TOTSUB]
    BSUBMAX = max(boffg[b + 1] - boffg[b] for b in range(NBLK))
    # first non-empty block per window
    first_b = [
        min(b for b in range(NBLK) if WS[w][b] > 0) for w in range(NWIN)
    ]
    GRPB = [round(g * NWIN / NGRP) for g in range(NGRP + 1)]
    GMAX = max(GRPB[g + 1] - GRPB[g] for g in range(NGRP))

    nc = bacc.Bacc(None, num_swdge_queues=4)
    hT_d = nc.declare_dram_parameter("hT", [IN_DIM, NPAD], BF16, isOutput=False)
    hTo_d = nc.declare_dram_parameter("hTo", [IN_DIM, NLOC], BF16, isOutput=False)
    Ws_d = nc.declare_dram_parameter("Ws", [IN_DIM, 264], BF16, isOutput=False)
    ident_d = nc.declare_dram_parameter("ident", [128, 128], BF16, isOutput=False)
    gidx_d = nc.declare_dram_parameter(
        "gidx", [128, TOTSUB * 8], I16, isOutput=False
    )
    indc_d = nc.declare_dram_parameter(
        "indc", [128, TOTSUB * 256], BF16, isOutput=False
    )
    out_d = nc.declare_dram_parameter("out", [NLOC, OUT_DIM], BF16, isOutput=True)

    # one table per gather block so block-b gathers only wait on their
    # own quarter of phase 1 (Tile tracks DRAM deps per tensor)
    tabs = [
        nc.dram_tensor(f"tabA{b}", [min(SBLK, NPAD - SBLK * b), BROW], BF16)
        for b in range(NBLK)
    ]

    with tile.TileContext(nc) as tc:
        with (
            tc.tile_pool(name="const", bufs=1) as cpool,
            tc.tile_pool(name="pgi", bufs=2) as pgi,
            tc.tile_pool(name="pind", bufs=2) as pind,
            tc.tile_pool(name="pG", bufs=3) as pG,
            tc.tile_pool(name="pGw", bufs=2) as pGw,
            tc.tile_pool(name="p1h", bufs=2) as p1h,
            tc.tile_pool(name="p1b", bufs=2) as p1b,
            tc.tile_pool(name="p1ps", bufs=2, space="PSUM") as p1ps,
            tc.tile_pool(name="p1st", bufs=2) as p1st,
            tc.tile_pool(name="pe", bufs=3) as pe,
            tc.tile_pool(name="pps", bufs=3, space="PSUM") as pps,
            tc.tile_pool(name="ppsE", bufs=1, space="PSUM") as ppsE,
            tc.tile_pool(name="p1bps", bufs=2, space="PSUM") as p1bps,
            tc.tile_pool(name="pno", bufs=1) as pno,
        ):
            Ws_t = cpool.tile_from(Ws_d[:])
            ident_t = cpool.tile_from(ident_d[:])
            s2st = cpool.tile([128, NWIN * 4], BF16)
            acc = cpool.tile([128, NWIN * USED], BF16)

            def load_block_idx(b):
                g0, g1 = boffg[b], boffg[b + 1]
                gi = pgi.tile([128, BSUBMAX * 8], I16, tag="gi")
                nc.sync.dma_start(
                    gi[:, 0 : (g1 - g0) * 8], gidx_d[:, g0 * 8 : g1 * 8]
                )
                return gi

            gi_b = load_block_idx(0)
            for _ in range(3):
                g0t = pG.tile([128, CWSMAX * BROW], BF16, tag="G")
                nc.gpsimd.memset(g0t[:], 0.0)
            # one-time: ones into cols 0:4 of each 16-row stb buffer
            for _ in range(2):
                stb0 = p1st.tile([128, 16 * BROW], BF16, tag="stb")
                nc.gpsimd.memset(
                    stb0[:].rearrange("p (i e) -> p i e", e=BROW)[:, :, 0:4],
                    1.0,
                )

            # ---- phase 1a chunk: haug = h @ Ws -> tabA (bf16 rows) ----
            def p1a_chunk(c):
                hc = p1h.tile([128, 2048], BF16, tag="hc")
                nc.sync.dma_start(hc[:], hT_d[:, c * 2048 : (c + 1) * 2048])
                stb = p1st.tile([128, 16 * BROW], BF16, tag="stb")
                for i in range(16):
                    ps = p1ps.tile([128, USED], F32, tag="ps")
                    nc.tensor.matmul(
                        ps[:],
                        hc[:, i * 128 : (i + 1) * 128],
                        Ws_t[:, 0:USED],
                        start=True,
                        stop=True,
                    )
                    # row: [0:4 ones (persist) | 4:260 hw | 260:264 s1]
                    dst_sl = stb[:, i * BROW + 4 : i * BROW + 264]
                    if i % 2 == 0:
                        nc.vector.tensor_copy(dst_sl, ps[:])
                    else:
                        nc.scalar.activation(
                            dst_sl, ps[:], mybir.ActivationFunctionType.Copy
                        )
                cb, co = c // CPB, (c % CPB) * 2048
                nc.sync.dma_start(
                    tabs[cb][co : co + 2048, :].rearrange(
                        "(i p) e -> p i e", p=128
                    ),
                    stb[:].rearrange("p (i e) -> p i e", e=BROW),
                )

            for c in range(CPB):
                p1a_chunk(c)
            pc = CPB

            # ---- phase 1b: s_dst for own dst range -> SBUF (s2st).
            # Emitted after the upfront p1a chunks so the table writes
            # (which gate the first gathers) hit the sync queue first;
            # hTo is loaded in a few batched DMAs, and the tiny matmul ->
            # copy chain pipelines through its own 2-deep PSUM pool. ----
            GB = 14
            for g0 in range(0, NWIN, GB):
                g1 = min(g0 + GB, NWIN)
                htw = p1b.tile([128, GB * 128], BF16, tag="htw")
                nc.sync.dma_start(
                    htw[:, 0 : (g1 - g0) * 128],
                    hTo_d[:, g0 * 128 : g1 * 128],
                )
                for t2 in range(g0, g1):
                    ps2 = p1bps.tile([128, 4], F32, tag="ps2")
                    nc.tensor.matmul(
                        ps2[:],
                        htw[:, (t2 - g0) * 128 : (t2 - g0 + 1) * 128],
                        Ws_t[:, 260:264],
                        start=True,
                        stop=True,
                    )
                    nc.scalar.activation(
                        s2st[:, t2 * 4 : (t2 + 1) * 4],
                        ps2[:],
                        mybir.ActivationFunctionType.Copy,
                    )

            # ---- phase 2: block-major chunked streams, interleaved with
            # the remaining phase-1a chunks (1 per 2 phase-2 chunks) ----
            cur_b = 0
            ci = 0
            qc = [0]
            gi_tiles = {0: gi_b}
            for b, w0, w1 in chunks:
                if b != cur_b:
                    cur_b = b
                gi_b = gi_tiles[b]
                # prefetch the next block's gather indices mid-block so
                # the block transition never stalls on the idx DMA
                if b + 1 < NBLK and b + 1 not in gi_tiles and w1 >= NWIN // 2:
                    gi_tiles[b + 1] = load_block_idx(b + 1)
                wss = [WS[w][b] for w in range(w0, w1)]
                cws = sum(wss)
                if cws == 0:
                    continue
                j0 = suboff[b][w0]
                jl = j0 - boffg[b]          # local subtile offset in block

                # indicator streams for this chunk from DRAM (interleaved)
                Ic = pind.tile([128, CWSMAX * 256], BF16, tag="Ic")
                nc.sync.dma_start(
                    Ic[:, 0 : cws * 256],
                    indc_d[:, j0 * 256 : (j0 + cws) * 256],
                )
                Ic4 = Ic[:, 0 : cws * 256].rearrange(
                    "p (s t c) -> p s t c", t=2, c=128
                )
                Ind3 = Ic4[:, :, 0, :]
                IndT3 = Ic4[:, :, 1, :]

                # gather per-edge rows (one call per region), round-robin
                # across the 4 SWDGE queues: queue q runs on Q7 core pair
                # (2q, 2q+1), so gathers on different queues generate
                # descriptors concurrently. Calls stay per-window to keep
                # each descriptor batch within the SWDGE ring capacity.
                G = pG.tile([128, CWSMAX * BROW], BF16, tag="G")
                G3 = G[:, 0 : cws * BROW].rearrange("p (s e) -> p s e", e=BROW)
                jg = 0
                for wi in range(len(wss)):
                    ws = wss[wi]
                    if ws == 0:
                        continue
                    mc = MC[w0 + wi][b]
                    nc.gpsimd.dma_gather(
                        G3[:, jg : jg + ws, :],
                        tabs[b][:, :],
                        gi_b[:, (jl + jg) * 8 : (jl + jg + ws) * 8],
                        mc,
                        mc,
                        BROW,
                        queue_num=qc[0] % 4,
                    )
                    qc[0] += 1
                    jg += ws

                # psE[slot, 4] = IndT_j.T @ swin(owner) + Ident.T @ s1_j
                # == full logit L per slot, accumulated on the PE
                psEt = ppsE.tile([128, CWSMAX * 12], F32, tag="psE", name="psE")
                pb0 = (ci % 3) * CWSMAX * 4
                ci += 1
                psE = psEt[:, pb0 : pb0 + CWSMAX * 4]
                j = 0
                for wi, w in enumerate(range(w0, w1)):
                    for _ in range(wss[wi]):
                        nc.tensor.matmul(
                            psE[:, 4 * j : 4 * j + 4],
                            IndT3[:, j, :],
                            s2st[:, 4 * w : 4 * w + 4],
                            start=True,
                            stop=False,
                        )
                        nc.tensor.matmul(
                            psE[:, 4 * j : 4 * j + 4],
                            ident_t[:],
                            G3[:, j, 260:264],
                            start=False,
                            stop=True,
                        )
                        j += 1

                # e = min(max(exp(L), exp(0.2 L)), HI); lo-clip never
                # binds (|logit| <= ~22 -> e >= 0.01)
                E1 = pe.tile([128, CWSMAX * 4], F32, tag="E1")
                nc.scalar.activation(
                    E1[:, 0 : cws * 4],
                    psE[:, 0 : cws * 4],
                    mybir.ActivationFunctionType.Exp,
                )
                E2 = pe.tile([128, CWSMAX * 4], F32, tag="E2")
                nc.scalar.activation(
                    E2[:, 0 : cws * 4],
                    psE[:, 0 : cws * 4],
                    mybir.ActivationFunctionType.Exp,
                    scale=SLOPE,
                )
                nc.vector.tensor_tensor(
                    E1[:, 0 : cws * 4],
                    E1[:, 0 : cws * 4],
                    E2[:, 0 : cws * 4],
                    MAX,
                )
                Eb = pe.tile([128, CWSMAX * 4], BF16, tag="Eb")
                nc.vector.tensor_scalar(
                    Eb[:, 0 : cws * 4], E1[:, 0 : cws * 4], CLIP_HI, None, MIN
                )

                # one fused weighting multiply:
                # Gw[s, 0:260] = G[s, 0:260] * rep65(e[s, 0:4])
                Gw = pGw.tile([128, CWSMAX * USED], BF16, tag="Gw")
                Gw4 = Gw[:, 0 : cws * USED].rearrange(
                    "p (s g f) -> p s g f", g=65, f=4
                )
                G4 = G[:, 0 : cws * BROW].rearrange(
                    "p (s e) -> p s e", e=BROW
                )[:, :, 0:USED].rearrange("p s (g f) -> p s g f", f=4)
                Eb4 = Eb[:, 0 : cws * 4].rearrange(
                    "p (s o f) -> p s o f", o=1, f=4
                )
                eb_b, g4_b = broadcast_tensor_aps(Eb4, G4)
                nc.vector.tensor_tensor(Gw4, g4_b, eb_b, MUL)
                Gw3 = Gw[:, 0 : cws * USED].rearrange(
                    "p (s e) -> p s e", e=USED
                )

                # per-window segment matmuls -> PSUM -> acc[w]
                j = 0
                for wi, w in enumerate(range(w0, w1)):
                    ws = wss[wi]
                    if ws == 0:
                        continue
                    aw = acc[:, w * USED : (w + 1) * USED]
                    ps = pps.tile([128, USED], F32, tag="mps")
                    for jj in range(j, j + ws):
                        nc.tensor.matmul(
                            ps[:],
                            Ind3[:, jj, :],
                            Gw3[:, jj, :],
                            start=(jj == j),
                            stop=(jj == j + ws - 1),
                        )
                    j += ws
                    if b == first_b[w]:
                        nc.vector.tensor_copy(aw, ps[:])
                    else:
                        nc.vector.tensor_tensor(aw, ps[:], aw, ADD)

                if pc < NPAD // 2048 and ci % 2 == 0:
                    p1a_chunk(pc)
                    pc += 1

            while pc < NPAD // 2048:
                p1a_chunk(pc)
                pc += 1

            # ---- batched final normalization over window groups ----
            acc3 = acc[:].rearrange("p (w e) -> p w e", e=USED)
            R = pno.tile([128, NWIN * 4], F32, tag="R")
            nc.vector.tensor_scalar(
                R[:].rearrange("p (w f) -> p w f", f=4),
                acc3[:, :, 0:4],
                1e-30,
                None,
                MAX,
            )
            R2 = pno.tile([128, NWIN * 4], F32, tag="R2")
            nc.vector.reciprocal(R2[:], R[:])
            nc.vector.tensor_scalar(R2[:], R2[:], 1.0 / HEADS, None, MUL)
            for g in range(NGRP):
                g0, g1 = GRPB[g], GRPB[g + 1]
                gw = g1 - g0
                accg = acc[:, g0 * USED : g1 * USED].rearrange(
                    "p (w e) -> p w e", e=USED
                )[:, :, 4:USED].rearrange("p w (j f) -> p w j f", f=4)
                r2g = R2[:, g0 * 4 : g1 * 4].rearrange(
                    "p (w o f) -> p w o f", o=1, f=4
                )
                Og = pno.tile([128, GMAX * OUT_DIM], BF16, tag="Og")
                Og3 = Og[:, 0 : gw * OUT_DIM].rearrange(
                    "p (w j) -> p w j", j=OUT_DIM
                )
                T = pno.tile([128, GMAX * OUT_DIM], BF16, tag="T")
                T3 = T[:, 0 : gw * OUT_DIM].rearrange(
                    "p (w j) -> p w j", j=OUT_DIM
                )
                for k in range(HEADS):
                    r2b, mb = broadcast_tensor_aps(
                        r2g[:, :, :, k], accg[:, :, :, k]
                    )
                    if k == 0:
                        nc.vector.tensor_tensor(Og3, mb, r2b, MUL)
                    else:
                        nc.vector.tensor_tensor(T3, mb, r2b, MUL)
                        nc.vector.tensor_tensor(Og3, Og3, T3, ADD)
                nc.sync.dma_start(
                    out_d[g0 * 128 : g1 * 128, :].rearrange(
                        "(w p) j -> p w j", p=128
                    ),
                    Og3,
                )

    nc.compile()
    _split_waits(nc)
    return nc


def kernel(h, edges, W, a):
    shared, per_core, WS, MC = _preprocess(h, edges, W, a)
    nc = _build(WS, MC)
    in_maps = [{**shared, **pc} for pc in per_core]
    r = run_bass_kernel_spmd(
        nc, in_maps, list(range(NC)), trace=bool(os.environ.get("GAT_TRACE"))
    )
    res = r.results
    global _last_results, _last_exec_ns, _last_bkr
    _last_results = res
    _last_exec_ns = r.exec_time_ns
    _last_bkr = r
    out = np.zeros((N, OUT_DIM), np.float32)
    for m in range(NC):
        out[m * NPC : (m + 1) * NPC] = res[m]["out"][:NPC].astype(np.float32)
    return out
